# revision 1
# baseline (speedup 1.0000x reference)
"""Trainium2 Bass kernel for nn_CAM (GNN message passing, 8-core SPMD).

Strategy (per core i of 8, owning node rows R_i = [1024*i, 1024*(i+1))):
  - Host ships the TRANSPOSED column-block of each adjacency:
    adjT_x[:, R_i] (shape [8192, 1024]) so the TensorE contraction axis
    (all 8192 source nodes) lands on SBUF partitions. fp32, streamed from
    HBM exactly once (adjT_f and adjT_s on the sync HWDGE ring; x/W1 on
    the scalar HWDGE ring so the two streams don't serialize).
  - The blend  con = meta*A_f + (1-meta)*A_s  is algebraically refactored
    as  con = (1-meta) * R  with  R = c*A_f + A_s,  c = meta/(1-meta).
    R is formed chunk-by-chunk with ONE fused DVE op
    (scalar_tensor_tensor: (A_f * c) + A_s -> bf16) and kept RESIDENT in
    SBUF (16 MB), so rounds 2 and 3 re-read it from SBUF instead of HBM.
    The (1-meta) factor is folded into the support matrices right before
    each AllGather, so no extra work on the big matrices is ever done.
  - Support matrices are computed in the transposed domain
    (z^T = s-stationary matmuls with R^T as the bf16 moving operand),
    exchanged across cores with AllGather (bf16) between rounds.
  - The attention fusion runs entirely in the transposed [64, 1024]
    domain; the host transposes the tiny per-core outputs back.
"""

import sys

if "/opt/trn_rl_repo" not in sys.path:
    sys.path.insert(0, "/opt/trn_rl_repo")

from contextlib import ExitStack

import numpy as np

import concourse.bass as bass
import concourse.tile as tile
from concourse import bacc, mybir
from concourse.bass_utils import run_bass_kernel_spmd
from concourse.masks import make_identity


F32 = mybir.dt.float32
BF16 = mybir.dt.bfloat16
AF = mybir.ActivationFunctionType

N = 8192
D_IN = 3000
H1, H2, Z = 256, 128, 64
N_CORES = 8
NL = N // N_CORES           # 1024 local nodes per core
KC = N // 128               # 64 contraction chunks of 128
XC = (D_IN + 127) // 128    # 24 x-feature chunks (last partial: 56)
RG = [list(range(N_CORES))]


def _emit(nc, tc, io):
    """Emit the whole per-core program inside a TileContext.

    Tile pools are a stack allocator (strict LIFO, zones reused across
    sibling scopes -- a later pool's first use waits for the release of
    the pool whose zone it reuses). The A-phase staging pool is opened
    BEFORE the phase-0 pools so the adjacency stream starts at t=0.

    Engine queues are FIFO, so DMA issue placement matters:
      sync ring   : adjT_f all slabs + adjT_s slabs < ACT_SPLIT
      scalar ring : x / W1 slabs (early), then s1-chain ACT compute,
                    then adjT_s slabs >= ACT_SPLIT, then tail chains
      gpsimd      : consts + AllGather bounces (naturally serial)
    """
    adjT_f, adjT_s, xT = io["adjT_f"], io["adjT_s"], io["xT"]
    zfT, zsT = io["zfT"], io["zsT"]
    W1, W2, W3 = io["W1"], io["W2"], io["W3"]
    wl_W, mlp_W = io["wl_W"], io["mlp_W"]
    wl_b, mlp_b, meta = io["wl_b"], io["mlp_b"], io["meta"]
    outT = io["outT"]

    ACT_SPLIT = 16  # adjT_s slabs >= this index go on the scalar ring

    ctx = ExitStack()
    with ctx:
        const = ctx.enter_context(tc.tile_pool(name="const", bufs=1))
        dram = ctx.enter_context(tc.tile_pool(name="dram", bufs=1, space="DRAM"))

        # ---- constants ----
        # meta comes first on the sync ring so the blend scalar is ready
        # before the first adjacency slab lands.
        meta_sb = const.tile([128, 1], F32)
        nc.sync.dma_start(out=meta_sb, in_=meta.ap().to_broadcast((128, 1)))
        om_sb = const.tile([128, 1], F32)    # 1 - meta
        nc.scalar.activation(om_sb, meta_sb, AF.Copy, scale=-1.0, bias=1.0)
        rec_om = const.tile([128, 1], F32)   # 1 / (1 - meta)
        nc.vector.reciprocal(rec_om, om_sb)
        c_sb = const.tile([128, 1], F32)     # meta / (1 - meta)
        nc.vector.tensor_mul(c_sb, meta_sb, rec_om)

        ident_bf = const.tile([128, 128], BF16)
        make_identity(nc, ident_bf)
        wl_b_sb = const.tile([64, 1], F32)
        nc.gpsimd.dma_start(out=wl_b_sb, in_=wl_b[:, None])
        mlp_b_sb = const.tile([64, 1], F32)
        nc.gpsimd.dma_start(out=mlp_b_sb, in_=mlp_b[:, None])
        W2_sb = const.tile([128, 2, H2], BF16)
        nc.gpsimd.dma_start(out=W2_sb, in_=W2.rearrange("(b k) c -> k b c", b=2))
        W3_sb = const.tile([128, Z], BF16)
        nc.gpsimd.dma_start(out=W3_sb, in_=W3[:, :])

        # resident blended adjacency (transposed): R^T[k_part, k_chunk, m]
        conT = const.tile([128, KC, NL], BF16)

        # AG bounce buffers
        s1_in = dram.tile([128, 8, H1], BF16)
        s1_out = dram.tile([N // 8, 8, H1], BF16, addr_space="Shared")
        # Support matrices are exchanged in the SWIZZLED on-chip layout
        # [128, 8, C] (partition p, node-chunk a, col c) so both the
        # AG-input writes and the gather reads are fully contiguous per
        # partition. AG concatenates on the first axis: the gathered
        # tensor is [8*128, 8, C] with core on the outer axis, and chunk
        # k of the stationary lives at [core=k//8][:, a=k%8, :].
        # s2/s3 are exchanged as two half-collectives so the second half
        # overlaps the first half's round matmuls
        s2_inA = dram.tile([128, 4, H2], BF16)
        s2_outA = dram.tile([N // 8, 4, H2], BF16, addr_space="Shared")
        s2_inB = dram.tile([128, 4, H2], BF16)
        s2_outB = dram.tile([N // 8, 4, H2], BF16, addr_space="Shared")
        s3_inA = dram.tile([128, 4, Z], BF16)
        s3_outA = dram.tile([N // 8, 4, Z], BF16, addr_space="Shared")
        s3_inB = dram.tile([128, 4, Z], BF16)
        s3_outB = dram.tile([N // 8, 4, Z], BF16, addr_space="Shared")

        # ======== phases 0+A+B: s1, stream+blend+round1, s2 ===========
        # Emission-order choreography (engine queues are in-order):
        #   scalar ring: x/W1 bf16 DMAs, then adjT_s slabs 6..31 -- the
        #     s1-chain ACT compute (tanh + scaled transpose-copies) is
        #     emitted between slabs 9 and 10 so it only stalls the ring
        #     for the ~2us it actually needs.
        #   sync ring: adjT_f all slabs + adjT_s slabs 0..5.
        #   PE: s1 matmuls, s1 transposes (at slab 10), then round-1
        #     matmuls emitted from slab 11 with catch-up pacing (4
        #     chunks/slab until they reach the blend frontier).
        with tc.tile_pool(name="tailB", bufs=1) as tailB, \
             tc.tile_pool(name="stage", bufs=2) as stage, \
             tc.tile_pool(name="psZ", bufs=1, space="PSUM") as psZ:
            z1sb = tailB.tile([128, 2, NL], BF16)
            z1_ps = [psZ.tile([128, NL], F32, name=f"z1g{g}") for g in range(2)]

            with tc.tile_pool(name="chain0", bufs=1) as chain0:
                s1T_bf = chain0.tile([128, 2, NL], BF16)
                s1loc = chain0.tile([128, 8, H1], BF16)
                with tc.tile_pool(name="xstage", bufs=3) as xstage:
                    psA_ctx = ExitStack()
                    psA = psA_ctx.enter_context(
                        tc.tile_pool(name="psA", bufs=1, space="PSUM")
                    )
                    s1T_ps = [psA.tile([128, NL], F32, name=f"s1T{g}")
                              for g in range(2)]
                    for kx in range(XC):
                        kp = min(128, D_IN - kx * 128)
                        xbf = xstage.tile([128, NL], BF16, name="xbf", bufs=6)
                        nc.scalar.dma_start(
                            out=xbf[:kp], in_=xT[kx * 128 : kx * 128 + kp, :]
                        )
                        w1bf = xstage.tile([128, H1], BF16, name="w1bf", bufs=6)
                        nc.scalar.dma_start(
                            out=w1bf[:kp], in_=W1[kx * 128 : kx * 128 + kp, :]
                        )
                        for g in range(2):
                            for h in range(2):
                                nc.tensor.matmul(
                                    s1T_ps[g][:, h * 512 : (h + 1) * 512],
                                    lhsT=w1bf[:kp, g * 128 : (g + 1) * 128],
                                    rhs=xbf[:kp, h * 512 : (h + 1) * 512],
                                    start=(kx == 0),
                                    stop=(kx == XC - 1),
                                )

                    # ---- phase A slab loop (s1 chain emitted at j==10) --
                    s1f = []
                    p1_next = 0

                    def emit_pass1_upto(limit, budget):
                        nonlocal p1_next
                        n = 0
                        while p1_next < limit and n < budget:
                            k = p1_next
                            for g in range(2):
                                for h in range(2):
                                    sl = slice(h * 512, (h + 1) * 512)
                                    nc.tensor.matmul(
                                        z1_ps[g][:, sl],
                                        lhsT=s1f[k // 8][:, k % 8,
                                                         g * 128 : (g + 1) * 128],
                                        rhs=conT[:, k, sl],
                                        start=(k == 0),
                                        stop=(k == KC - 1),
                                    )
                            p1_next += 1
                            n += 1

                    for j in range(KC // 2):  # 32 slabs x 2 chunks
                        if j == 10:
                            # s1 chain: tanh -> scaled transposes -> AG
                            for g in range(2):
                                nc.scalar.activation(
                                    s1T_bf[:, g], s1T_ps[g], AF.Tanh
                                )
                            psA_ctx.close()
                            with tc.tile_pool(
                                name="psT", bufs=2, space="PSUM"
                            ) as psT:
                                for mb in range(8):
                                    for g in range(2):
                                        tp = psT.tile([128, 128], BF16, name="tp")
                                        nc.tensor.transpose(
                                            tp,
                                            s1T_bf[:, g, mb * 128 : (mb + 1) * 128],
                                            ident_bf,
                                        )
                                        nc.scalar.activation(
                                            s1loc[:, mb, g * 128 : (g + 1) * 128],
                                            tp, AF.Copy, scale=om_sb,
                                        )
                            nc.gpsimd.dma_start(out=s1_in[:, :, :], in_=s1loc)
                            nc.gpsimd.collective_compute(
                                "AllGather", mybir.AluOpType.bypass,
                                replica_groups=RG,
                                ins=[s1_in.opt()], outs=[s1_out.opt()],
                            )
                            for q in range(8):
                                s1f_q = stage.tile([128, 8, H1], BF16,
                                                   name="s1f", bufs=3)
                                nc.gpsimd.dma_start(
                                    out=s1f_q,
                                    in_=s1_out[q * 128 : (q + 1) * 128],
                                )
                                s1f.append(s1f_q)
                        af = stage.tile([128, 2, NL], F32, name="af")
                        nc.sync.dma_start(
                            out=af,
                            in_=adjT_f[j * 256 : (j + 1) * 256, :].rearrange(
                                "(a p) m -> p a m", p=128
                            ),
                        )
                        asl = stage.tile([128, 2, NL], F32, name="asl")
                        as_eng = nc.sync if j < 6 else nc.scalar
                        as_eng.dma_start(
                            out=asl,
                            in_=adjT_s[j * 256 : (j + 1) * 256, :].rearrange(
                                "(a p) m -> p a m", p=128
                            ),
                        )
                        for t in range(2):
                            k = 2 * j + t
                            nc.vector.scalar_tensor_tensor(
                                out=conT[:, k, :],
                                in0=af[:, t, :],
                                scalar=c_sb,
                                in1=asl[:, t, :],
                                op0=mybir.AluOpType.mult,
                                op1=mybir.AluOpType.add,
                            )
                        if j >= 11:
                            emit_pass1_upto(limit=2 * j, budget=4)
                    emit_pass1_upto(limit=KC, budget=KC)
            # copy z1 out of PSUM (bf16) before psZ closes
            nc.vector.tensor_copy(z1sb[:, 0], z1_ps[0])
            nc.scalar.copy(z1sb[:, 1], z1_ps[1])

        # ---- phase B: s2 = tanh(z1 @ W2) ----
        with tc.tile_pool(name="tailB2", bufs=1) as tailB2:
            s2T_bf2 = tailB2.tile([128, NL], BF16)
            s2loc2 = tailB2.tile([128, 8, H2], BF16)
            with tc.tile_pool(name="psC", bufs=1, space="PSUM") as psC:
                s2T_ps = psC.tile([128, NL], F32)
                for b in range(2):
                    for h in range(2):
                        sl = slice(h * 512, (h + 1) * 512)
                        nc.tensor.matmul(
                            s2T_ps[:, sl], lhsT=W2_sb[:, b], rhs=z1sb[:, b, sl],
                            start=(b == 0), stop=(b == 1),
                        )
                nc.scalar.activation(s2T_bf2, s2T_ps, AF.Tanh)
            with tc.tile_pool(name="psT2", bufs=2, space="PSUM") as psT2:
                for mb in range(8):
                    tp = psT2.tile([128, 128], BF16, name="tp2")
                    nc.tensor.transpose(
                        tp, s2T_bf2[:, mb * 128 : (mb + 1) * 128], ident_bf
                    )
                    nc.scalar.activation(s2loc2[:, mb], tp, AF.Copy, scale=om_sb)
                    if mb == 3:
                        nc.gpsimd.dma_start(out=s2_inA[:, :, :],
                                            in_=s2loc2[:, 0:4])
                        nc.gpsimd.collective_compute(
                            "AllGather", mybir.AluOpType.bypass,
                            replica_groups=RG,
                            ins=[s2_inA.opt()], outs=[s2_outA.opt()],
                        )
            nc.gpsimd.dma_start(out=s2_inB[:, :, :], in_=s2loc2[:, 4:8])
            nc.gpsimd.collective_compute(
                "AllGather", mybir.AluOpType.bypass, replica_groups=RG,
                ins=[s2_inB.opt()], outs=[s2_outB.opt()],
            )

        # ================= phase C: round 2 (z2 = R @ s2') =============
        with tc.tile_pool(name="tailC", bufs=1) as tailC:
            z2sb = tailC.tile([128, NL], BF16)
            s3T_bf = tailC.tile([64, NL], BF16)
            s3loc = tailC.tile([128, 8, Z], BF16)
            with tc.tile_pool(name="stageC", bufs=4) as stageC, \
                 tc.tile_pool(name="psD", bufs=1, space="PSUM") as psD:
                z2_ps = psD.tile([128, NL], F32)
                s2fA, s2fB = [], []
                for q in range(8):
                    s2f_q = stageC.tile([128, 4, H2], BF16, name="s2fA", bufs=8)
                    nc.scalar.dma_start(
                        out=s2f_q, in_=s2_outA[q * 128 : (q + 1) * 128]
                    )
                    s2fA.append(s2f_q)
                for q in range(8):
                    s2f_q = stageC.tile([128, 4, H2], BF16, name="s2fB", bufs=8)
                    nc.scalar.dma_start(
                        out=s2f_q, in_=s2_outB[q * 128 : (q + 1) * 128]
                    )
                    s2fB.append(s2f_q)
                korder = [8 * q + a for half in (0, 1) for q in range(8)
                          for a in range(4 * half, 4 * half + 4)]
                for i, k in enumerate(korder):
                    q, a = k // 8, k % 8
                    lhsT = s2fA[q][:, a, :] if a < 4 else s2fB[q][:, a - 4, :]
                    for h in range(2):
                        sl = slice(h * 512, (h + 1) * 512)
                        nc.tensor.matmul(
                            z2_ps[:, sl], lhsT=lhsT, rhs=conT[:, k, sl],
                            start=(i == 0), stop=(i == KC - 1),
                        )
                nc.vector.tensor_copy(z2sb[:, :512], z2_ps[:, :512])
                nc.scalar.copy(z2sb[:, 512:], z2_ps[:, 512:])

            # s3 = z2 @ W3 (no activation); scale by (1-meta) in the copy
            with tc.tile_pool(name="psE", bufs=1, space="PSUM") as psE:
                s3T_ps = psE.tile([64, NL], F32)
                for h in range(2):
                    sl = slice(h * 512, (h + 1) * 512)
                    nc.tensor.matmul(s3T_ps[:, sl], lhsT=W3_sb, rhs=z2sb[:, sl])
                nc.scalar.copy(s3T_bf, s3T_ps)
            with tc.tile_pool(name="psT3", bufs=2, space="PSUM") as psT3:
                for mb in range(8):
                    tp = psT3.tile([128, 64], BF16, name="tp3")
                    nc.tensor.transpose(
                        tp, s3T_bf[:, mb * 128 : (mb + 1) * 128], ident_bf[:64, :64]
                    )
                    nc.scalar.activation(s3loc[:, mb], tp, AF.Copy, scale=om_sb)
                    if mb == 3:
                        nc.gpsimd.dma_start(out=s3_inA[:, :, :],
                                            in_=s3loc[:, 0:4])
                        nc.gpsimd.collective_compute(
                            "AllGather", mybir.AluOpType.bypass,
                            replica_groups=RG,
                            ins=[s3_inA.opt()], outs=[s3_outA.opt()],
                        )
            nc.gpsimd.dma_start(out=s3_inB[:, :, :], in_=s3loc[:, 4:8])
            nc.gpsimd.collective_compute(
                "AllGather", mybir.AluOpType.bypass, replica_groups=RG,
                ins=[s3_inB.opt()], outs=[s3_outB.opt()],
            )

        # ========= phases D+E: round 3 (com = R @ s3') + fusion ========
        # The com-independent attention matmuls (views 0 and 2) are
        # interleaved into the round-3 k-loop to keep the PE warm and
        # shorten the serial fusion tail.
        with tc.tile_pool(name="tailD", bufs=1) as tailD:
            comT = tailD.tile([64, NL], F32)
            zfT_sb = tailD.tile([64, NL], F32)
            nc.gpsimd.dma_start(out=zfT_sb, in_=zfT[:, :])
            zsT_sb = tailD.tile([64, NL], F32)
            nc.gpsimd.dma_start(out=zsT_sb, in_=zsT[:, :])
            wlW_sb = tailD.tile([64, 64], F32)
            nc.gpsimd.dma_start(out=wlW_sb, in_=wl_W[:, :])
            mlpW_sb = tailD.tile([64, 3, 64], F32)
            nc.gpsimd.dma_start(
                out=mlpW_sb, in_=mlp_W.rearrange("(v c) d -> c v d", v=3)
            )
            with tc.tile_pool(name="psG", bufs=2, space="PSUM") as psG:
                embs = [zfT_sb, comT, zsT_sb]
                aTs = [None, None, None]
                sqs = [None, None, None]

                def emit_attn_view(v):
                    a_ps = psG.tile([64, NL], F32, name="aps")
                    for h in range(2):
                        sl = slice(h * 512, (h + 1) * 512)
                        nc.tensor.matmul(a_ps[:, sl], lhsT=wlW_sb, rhs=embs[v][:, sl])
                    aT = tailD.tile([64, NL], F32, name=f"aT{v}")
                    nc.vector.tensor_scalar_add(aT, a_ps, wl_b_sb)
                    aTs[v] = aT
                    sqv = tailD.tile([64, NL], F32, name=f"sq{v}")
                    nc.scalar.activation(sqv, aT, AF.Square)
                    sqs[v] = sqv

                with tc.tile_pool(name="stageD", bufs=2) as stageD, \
                     tc.tile_pool(name="psF", bufs=1, space="PSUM") as psF:
                    com_ps = psF.tile([64, NL], F32)
                    s3fA, s3fB = [], []
                    for q in range(8):
                        s3f_q = stageD.tile([128, 4, Z], BF16, name="s3fA",
                                            bufs=8)
                        nc.scalar.dma_start(
                            out=s3f_q, in_=s3_outA[q * 128 : (q + 1) * 128]
                        )
                        s3fA.append(s3f_q)
                    for q in range(8):
                        s3f_q = stageD.tile([128, 4, Z], BF16, name="s3fB",
                                            bufs=8)
                        nc.scalar.dma_start(
                            out=s3f_q, in_=s3_outB[q * 128 : (q + 1) * 128]
                        )
                        s3fB.append(s3f_q)
                    korder = [8 * q + a for half in (0, 1) for q in range(8)
                              for a in range(4 * half, 4 * half + 4)]
                    for i, k in enumerate(korder):
                        if i == 44:
                            emit_attn_view(0)
                        if i == 54:
                            emit_attn_view(2)
                        q, a = k // 8, k % 8
                        lhsT = (s3fA[q][:, a, :] if a < 4
                                else s3fB[q][:, a - 4, :])
                        for h in range(2):
                            sl = slice(h * 512, (h + 1) * 512)
                            nc.tensor.matmul(
                                com_ps[:, sl], lhsT=lhsT, rhs=conT[:, k, sl],
                                start=(i == 0), stop=(i == KC - 1),
                            )
                    nc.vector.tensor_copy(comT, com_ps)

                # phase E: remaining fusion (com-dependent)
                emit_attn_view(1)
                sq = tailD.tile([64, NL], F32)
                nc.vector.tensor_add(sq, sqs[0], sqs[2])
                nc.vector.tensor_add(sq, sq, sqs[1])
                nrm = tailD.tile([64, NL], F32)
                nc.scalar.activation(nrm, sq, AF.Sqrt)
                nc.vector.tensor_scalar_max(nrm, nrm, 1e-12)
                rec = tailD.tile([64, NL], F32)
                nc.vector.reciprocal_approx_fast(rec, nrm)

                out_ps = psG.tile([64, NL], F32, name="ops", bufs=1)
                for v in range(3):
                    u = tailD.tile([64, NL], F32, name="u", bufs=2)
                    nc.vector.tensor_mul(u, aTs[v], rec)
                    nc.vector.tensor_mul(u, u, embs[v])
                    for h in range(2):
                        sl = slice(h * 512, (h + 1) * 512)
                        nc.tensor.matmul(
                            out_ps[:, sl], lhsT=mlpW_sb[:, v], rhs=u[:, sl],
                            start=(v == 0), stop=(v == 2),
                        )
                outT_sb = tailD.tile([64, NL], F32)
                nc.vector.tensor_scalar_add(outT_sb, out_ps, mlp_b_sb)
                nc.gpsimd.dma_start(out=outT[:, :], in_=outT_sb)


_CACHE = {}


def _build():
    if "nc" in _CACHE:
        return _CACHE["nc"]
    nc = bacc.Bacc("TRN2", target_bir_lowering=False, debug=False,
                   num_devices=N_CORES)
    io = {
        "adjT_f": nc.dram_tensor("adjT_f", [N, NL], F32, kind="ExternalInput"),
        "adjT_s": nc.dram_tensor("adjT_s", [N, NL], F32, kind="ExternalInput"),
        "xT": nc.dram_tensor("xT", [D_IN, NL], BF16, kind="ExternalInput"),
        "zfT": nc.dram_tensor("zfT", [Z, NL], F32, kind="ExternalInput"),
        "zsT": nc.dram_tensor("zsT", [Z, NL], F32, kind="ExternalInput"),
        "W1": nc.dram_tensor("W1", [D_IN, H1], BF16, kind="ExternalInput"),
        "W2": nc.dram_tensor("W2", [H1, H2], F32, kind="ExternalInput"),
        "W3": nc.dram_tensor("W3", [H2, Z], F32, kind="ExternalInput"),
        "wl_W": nc.dram_tensor("wl_W", [Z, Z], F32, kind="ExternalInput"),
        "mlp_W": nc.dram_tensor("mlp_W", [3 * Z, Z], F32, kind="ExternalInput"),
        "wl_b": nc.dram_tensor("wl_b", [Z], F32, kind="ExternalInput"),
        "mlp_b": nc.dram_tensor("mlp_b", [Z], F32, kind="ExternalInput"),
        "meta": nc.dram_tensor("meta", [1], F32, kind="ExternalInput"),
        "outT": nc.dram_tensor("outT", [Z, NL], F32, kind="ExternalOutput"),
    }
    with tile.TileContext(nc) as tc:
        _emit(nc, tc, io)
    nc.compile()
    _CACHE["nc"] = nc
    return nc


def _shard_inputs(inputs):
    """Full inputs -> per-core input maps (host-side sharding only)."""
    f32 = np.float32
    adj_f = np.asarray(inputs["adj_feature"], f32)
    adj_s = np.asarray(inputs["adj_spatial"], f32)
    x = np.asarray(inputs["x"], f32)
    zf = np.asarray(inputs["z_feature"], f32)
    zs = np.asarray(inputs["z_spatial"], f32)
    import ml_dtypes
    bf16 = ml_dtypes.bfloat16
    rep = {
        "W1": np.ascontiguousarray(np.asarray(inputs["W1"], f32).astype(bf16)),
        "W2": np.ascontiguousarray(np.asarray(inputs["W2"], f32)),
        "W3": np.ascontiguousarray(np.asarray(inputs["W3"], f32)),
        "wl_W": np.ascontiguousarray(np.asarray(inputs["wl_W"], f32)),
        "mlp_W": np.ascontiguousarray(np.asarray(inputs["mlp_W"], f32)),
        "wl_b": np.ascontiguousarray(np.asarray(inputs["wl_b"], f32)),
        "mlp_b": np.ascontiguousarray(np.asarray(inputs["mlp_b"], f32)),
        "meta": np.ascontiguousarray(np.asarray(inputs["meta"], f32)),
    }
    adj_fT = np.ascontiguousarray(adj_f.T)
    adj_sT = np.ascontiguousarray(adj_s.T)
    xT = np.ascontiguousarray(x.T)
    zfT = np.ascontiguousarray(zf.T)
    zsT = np.ascontiguousarray(zs.T)
    in_maps = []
    for i in range(N_CORES):
        r = slice(NL * i, NL * (i + 1))
        m = {
            "adjT_f": np.ascontiguousarray(adj_fT[:, r]),
            "adjT_s": np.ascontiguousarray(adj_sT[:, r]),
            "xT": np.ascontiguousarray(xT[:, r]).astype(bf16),
            "zfT": np.ascontiguousarray(zfT[:, r]),
            "zsT": np.ascontiguousarray(zsT[:, r]),
        }
        m.update(rep)
        in_maps.append(m)
    return in_maps


def run(trace=False, **inputs):
    nc = _build()
    in_maps = _shard_inputs(inputs)
    res = run_bass_kernel_spmd(nc, in_maps, list(range(N_CORES)), trace=trace)
    out = np.concatenate(
        [np.asarray(res.results[i]["outT"]).T for i in range(N_CORES)], axis=0
    ).astype(np.float32)
    return out, res


def kernel(**inputs):
    out, _ = run(trace=False, **inputs)
    return out



# revision 2
# speedup vs baseline: 1.5425x; 1.5425x over previous
"""Trainium2 Bass kernel for nn_CAM (GNN message passing, 8-core SPMD).

Strategy (per core i of 8, owning node rows R_i = [1024*i, 1024*(i+1))):
  - Host ships the TRANSPOSED column-block of each adjacency as
    fp8_e4m3 with a fixed 2^13 exponent shift (adj values are uniform
    [0, 1/8192]; the shift moves them into fp8's normal range and is
    folded back on-device via alpha = (1-meta)/2^13). This quarters
    the dominant HBM traffic vs f32 (8 MB + 8 MB per core).
  - x / W1 ship in bf16 and are issued at the HEAD of both DMA rings so
    the s1 = tanh(x@W1) chain and its AllGather start ~25 us in.
  - The blend  con = meta*A_f + (1-meta)*A_s  is refactored as
    con = (1-meta)/2^13 * R8  with  R8 = (c*A_f8 + A_s8),
    c = meta/(1-meta).  R8 is formed slab-by-slab with ONE fused DVE op
    (scalar_tensor_tensor, fp8 in -> fp8 out) and kept RESIDENT in SBUF
    (8 MB), so rounds 2 and 3 re-read it from SBUF instead of HBM.
  - All three adj@support rounds run as fp8 DoubleRow matmuls (2 packed
    contraction rows per PE pass = 2x throughput): lhsT is a [128,2,H]
    pair of gathered support chunks (fp8), rhs a [128,2,512] pair of
    resident R8 chunks. The support matrices are exchanged across cores
    in fp8 (AllGather bounced through shared DRAM), halving collective
    traffic; s2/s3 are exchanged as two half-collectives so the second
    half overlaps the first half's round matmuls.
  - The 2^-13*(1-meta) fold-back rides existing ACT ops for free (tanh
    input scale for s2, copy scales for s3 / com).
  - The attention fusion runs entirely in the transposed [64, 1024]
    domain; the host transposes the tiny per-core outputs back.
"""

import sys

if "/opt/trn_rl_repo" not in sys.path:
    sys.path.insert(0, "/opt/trn_rl_repo")

from contextlib import ExitStack

import numpy as np

import concourse.bass as bass
import concourse.tile as tile
from concourse import bacc, mybir
from concourse.bass_utils import run_bass_kernel_spmd
from concourse.masks import make_identity


F32 = mybir.dt.float32
BF16 = mybir.dt.bfloat16
F8 = mybir.dt.float8e4
AF = mybir.ActivationFunctionType
DR = mybir.MatmulPerfMode.DoubleRow

N = 8192
D_IN = 3000
H1, H2, Z = 256, 128, 64
N_CORES = 8
NL = N // N_CORES           # 1024 local nodes per core
KC = N // 128               # 64 contraction chunks of 128
NSLAB = 8                   # 8 slabs x 8 chunks for the adjacency stream
SCH = KC // NSLAB           # chunks per slab
XC = (D_IN + 127) // 128    # 24 x-feature chunks (last partial: 56)
RG = [list(range(N_CORES))]
INV13 = 1.0 / 8192.0        # 2^-13 fold-back for the fp8 exponent shift


def _emit(nc, tc, io):
    """Emit the whole per-core program inside a TileContext.

    Engine queues are FIFO; issue placement:
      sync ring   : meta, x/W1 even chunks, adjT_f slabs, AG input writes
      scalar ring : x/W1 odd chunks, 2 adjT_s slabs, s1 tanh chain,
                    adjT_s slabs 2.., then all tail ACT compute + reloads
      vector      : blend STTs (slab-paced), PSUM copies, fusion math
      gpsimd      : consts, AllGathers + their SBUF reloads, output
    """
    adjT_f, adjT_s, xT = io["adjT_f"], io["adjT_s"], io["xT"]
    zfT, zsT = io["zfT"], io["zsT"]
    W1, W2, W3 = io["W1"], io["W2"], io["W3"]
    wl_W, mlp_W = io["wl_W"], io["mlp_W"]
    wl_b, mlp_b, meta = io["wl_b"], io["mlp_b"], io["meta"]
    outT = io["outT"]

    ctx = ExitStack()
    with ctx:
        const = ctx.enter_context(tc.tile_pool(name="const", bufs=1))
        dram = ctx.enter_context(tc.tile_pool(name="dram", bufs=1, space="DRAM"))

        # ---- constants ----
        meta_sb = const.tile([128, 1], F32)
        nc.sync.dma_start(out=meta_sb, in_=meta.ap().to_broadcast((128, 1)))
        om_sb = const.tile([128, 1], F32)    # 1 - meta
        nc.scalar.activation(om_sb, meta_sb, AF.Copy, scale=-1.0, bias=1.0)
        alpha_sb = const.tile([128, 1], F32)  # (1 - meta) / 2^13
        nc.scalar.activation(alpha_sb, om_sb, AF.Copy, scale=INV13)
        rec_om = const.tile([128, 1], F32)   # 1 / (1 - meta)
        nc.vector.reciprocal(rec_om, om_sb)
        c_sb = const.tile([128, 1], F32)     # meta / (1 - meta)
        nc.vector.tensor_mul(c_sb, meta_sb, rec_om)

        ident_bf = const.tile([128, 128], BF16)
        make_identity(nc, ident_bf)
        wl_b_sb = const.tile([64, 1], F32)
        nc.gpsimd.dma_start(out=wl_b_sb, in_=wl_b[:, None])
        mlp_b_sb = const.tile([64, 1], F32)
        nc.gpsimd.dma_start(out=mlp_b_sb, in_=mlp_b[:, None])
        W2_sb = const.tile([128, 2, H2], BF16)
        nc.gpsimd.dma_start(out=W2_sb, in_=W2.rearrange("(b k) c -> k b c", b=2))
        W3_sb = const.tile([128, Z], BF16)
        nc.gpsimd.dma_start(out=W3_sb, in_=W3[:, :])

        # resident blended adjacency (transposed, fp8, x2^13):
        # R8[k_part, k_chunk, m]
        conT8 = const.tile([128, KC, NL], F8)
        # z1 (raw PSUM magnitude, bf16) lives across phases A->B
        z1sb = const.tile([128, 2, NL], BF16)

        # AG bounce buffers (fp8 payloads)
        s1_in = dram.tile([128, 8, H1], F8)
        s1_out = dram.tile([N // 8, 8, H1], F8, addr_space="Shared")
        s2_inA = dram.tile([128, 4, H2], F8)
        s2_outA = dram.tile([N // 8, 4, H2], F8, addr_space="Shared")
        s2_inB = dram.tile([128, 4, H2], F8)
        s2_outB = dram.tile([N // 8, 4, H2], F8, addr_space="Shared")
        s3_inA = dram.tile([128, 4, Z], F8)
        s3_outA = dram.tile([N // 8, 4, Z], F8, addr_space="Shared")
        s3_inB = dram.tile([128, 4, Z], F8)
        s3_outB = dram.tile([N // 8, 4, Z], F8, addr_space="Shared")

        # ======== phase A: stream+blend, s1 chain, round 1 ============
        with tc.tile_pool(name="phaseA", bufs=1) as pA, \
             tc.tile_pool(name="psZ", bufs=1, space="PSUM") as psZ:
            z1_ps = [psZ.tile([128, NL], F32, name=f"z1g{g}") for g in range(2)]
            s1T_bf = pA.tile([128, 2, NL], BF16)
            s1loc = pA.tile([128, 8, H1], F8)
            s1f = pA.tile([128, 8, 8, H1], F8)

            psA_ctx = ExitStack()
            psA = psA_ctx.enter_context(
                tc.tile_pool(name="psA", bufs=1, space="PSUM")
            )
            s1T_ps = [psA.tile([128, NL], F32, name=f"s1T{g}") for g in range(2)]

            # ---- x/W1 at the head of BOTH rings, alternating ----
            for kx in range(XC):
                kp = min(128, D_IN - kx * 128)
                eng = nc.sync if kx % 2 == 0 else nc.scalar
                xbf = pA.tile([128, NL], BF16, name="xbf", bufs=6)
                eng.dma_start(out=xbf[:kp], in_=xT[kx * 128 : kx * 128 + kp, :])
                w1bf = pA.tile([128, H1], BF16, name="w1bf", bufs=6)
                eng.dma_start(out=w1bf[:kp], in_=W1[kx * 128 : kx * 128 + kp, :])
                for g in range(2):
                    for h in range(2):
                        nc.tensor.matmul(
                            s1T_ps[g][:, h * 512 : (h + 1) * 512],
                            lhsT=w1bf[:kp, g * 128 : (g + 1) * 128],
                            rhs=xbf[:kp, h * 512 : (h + 1) * 512],
                            start=(kx == 0),
                            stop=(kx == XC - 1),
                        )

            # ---- adjacency slab loop (1 MB fp8 slabs, 8 chunks) ----
            for j in range(NSLAB):
                af = pA.tile([128, SCH, NL], F8, name="af", bufs=4)
                nc.sync.dma_start(
                    out=af,
                    in_=adjT_f[j * SCH * 128 : (j + 1) * SCH * 128, :].rearrange(
                        "(a p) m -> p a m", p=128
                    ),
                )
                asl = pA.tile([128, SCH, NL], F8, name="asl", bufs=4)
                nc.scalar.dma_start(
                    out=asl,
                    in_=adjT_s[j * SCH * 128 : (j + 1) * SCH * 128, :].rearrange(
                        "(a p) m -> p a m", p=128
                    ),
                )
                # one fused blend per slab: R8 = (af * c) + asl -> fp8
                nc.vector.scalar_tensor_tensor(
                    out=conT8[:, j * SCH : (j + 1) * SCH, :],
                    in0=af,
                    scalar=c_sb,
                    in1=asl,
                    op0=mybir.AluOpType.mult,
                    op1=mybir.AluOpType.add,
                )
                if j == 1:
                    # s1 chain: tanh -> transposes -> fp8 copies -> AG
                    for g in range(2):
                        nc.scalar.activation(s1T_bf[:, g], s1T_ps[g], AF.Tanh)
                    psA_ctx.close()
                    with tc.tile_pool(name="psT", bufs=2, space="PSUM") as psT:
                        for mb in range(8):
                            for g in range(2):
                                tp = psT.tile([128, 128], BF16, name="tp")
                                nc.tensor.transpose(
                                    tp,
                                    s1T_bf[:, g, mb * 128 : (mb + 1) * 128],
                                    ident_bf,
                                )
                                nc.scalar.activation(
                                    s1loc[:, mb, g * 128 : (g + 1) * 128],
                                    tp, AF.Copy,
                                )
                    nc.sync.dma_start(out=s1_in[:, :, :], in_=s1loc)
                    nc.gpsimd.collective_compute(
                        "AllGather", mybir.AluOpType.bypass,
                        replica_groups=RG,
                        ins=[s1_in.opt()], outs=[s1_out.opt()],
                    )
                    nc.gpsimd.dma_start(
                        out=s1f,
                        in_=s1_out.rearrange("(q p) a c -> p q a c", p=128),
                    )

            # ---- round 1: z1 = R8 @ s1q, fp8 DoubleRow over 32 pairs ----
            for jp in range(KC // 2):
                q, a = (2 * jp) // 8, (2 * jp) % 8
                for g in range(2):
                    for h in range(2):
                        sl = slice(h * 512, (h + 1) * 512)
                        nc.tensor.matmul(
                            z1_ps[g][:, sl],
                            lhsT=s1f[:, q, a : a + 2, g * 128 : (g + 1) * 128],
                            rhs=conT8[:, 2 * jp : 2 * jp + 2, sl],
                            start=(jp == 0),
                            stop=(jp == KC // 2 - 1),
                            perf_mode=DR,
                        )
            # z1 out of PSUM (raw magnitude; alpha folds in at s2's tanh)
            nc.vector.tensor_copy(z1sb[:, 0], z1_ps[0])
            nc.scalar.copy(z1sb[:, 1], z1_ps[1])

        # ======== phase B: s2 = tanh(alpha * z1 @ W2), AG ==============
        with tc.tile_pool(name="tailB", bufs=1) as tailB:
            s2T_bf = tailB.tile([128, NL], BF16)
            s2loc = tailB.tile([128, 8, H2], F8)
            with tc.tile_pool(name="psC", bufs=1, space="PSUM") as psC:
                s2T_ps = psC.tile([128, NL], F32)
                for b in range(2):
                    for h in range(2):
                        sl = slice(h * 512, (h + 1) * 512)
                        nc.tensor.matmul(
                            s2T_ps[:, sl], lhsT=W2_sb[:, b], rhs=z1sb[:, b, sl],
                            start=(b == 0), stop=(b == 1),
                        )
                nc.scalar.activation(s2T_bf, s2T_ps, AF.Tanh, scale=alpha_sb)
            with tc.tile_pool(name="psT2", bufs=2, space="PSUM") as psT2:
                for mb in range(8):
                    tp = psT2.tile([128, 128], BF16, name="tp2")
                    nc.tensor.transpose(
                        tp, s2T_bf[:, mb * 128 : (mb + 1) * 128], ident_bf
                    )
                    nc.scalar.activation(s2loc[:, mb], tp, AF.Copy)
                    if mb == 3:
                        nc.sync.dma_start(out=s2_inA[:, :, :], in_=s2loc[:, 0:4])
                        nc.gpsimd.collective_compute(
                            "AllGather", mybir.AluOpType.bypass,
                            replica_groups=RG,
                            ins=[s2_inA.opt()], outs=[s2_outA.opt()],
                        )
            nc.sync.dma_start(out=s2_inB[:, :, :], in_=s2loc[:, 4:8])
            nc.gpsimd.collective_compute(
                "AllGather", mybir.AluOpType.bypass, replica_groups=RG,
                ins=[s2_inB.opt()], outs=[s2_outB.opt()],
            )

        # ======== phase C: round 2 (z2 = R8 @ s2q), s3 chain ===========
        with tc.tile_pool(name="tailC", bufs=1) as tailC:
            z2sb = tailC.tile([128, NL], BF16)
            s3T_bf = tailC.tile([64, NL], BF16)
            s3loc = tailC.tile([128, 8, Z], F8)
            s2f = tailC.tile([128, 8, 8, H2], F8)
            with tc.tile_pool(name="psD", bufs=1, space="PSUM") as psD:
                z2_ps = psD.tile([128, NL], F32)
                nc.scalar.dma_start(
                    out=s2f[:, :, 0:4, :],
                    in_=s2_outA.rearrange("(q p) a c -> p q a c", p=128),
                )
                nc.scalar.dma_start(
                    out=s2f[:, :, 4:8, :],
                    in_=s2_outB.rearrange("(q p) a c -> p q a c", p=128),
                )
                porder = [(q, a) for half in (0, 1) for q in range(8)
                          for a in (4 * half, 4 * half + 2)]
                for i, (q, a) in enumerate(porder):
                    k = 8 * q + a
                    for h in range(2):
                        sl = slice(h * 512, (h + 1) * 512)
                        nc.tensor.matmul(
                            z2_ps[:, sl],
                            lhsT=s2f[:, q, a : a + 2, :],
                            rhs=conT8[:, k : k + 2, sl],
                            start=(i == 0),
                            stop=(i == KC // 2 - 1),
                            perf_mode=DR,
                        )
                nc.vector.tensor_copy(z2sb[:, :512], z2_ps[:, :512])
                nc.scalar.copy(z2sb[:, 512:], z2_ps[:, 512:])

            # s3 = alpha * (z2 @ W3); fold alpha into the PSUM copy
            with tc.tile_pool(name="psE", bufs=1, space="PSUM") as psE:
                s3T_ps = psE.tile([64, NL], F32)
                for h in range(2):
                    sl = slice(h * 512, (h + 1) * 512)
                    nc.tensor.matmul(s3T_ps[:, sl], lhsT=W3_sb, rhs=z2sb[:, sl])
                nc.scalar.activation(s3T_bf, s3T_ps, AF.Copy,
                                     scale=alpha_sb[:64])
            with tc.tile_pool(name="psT3", bufs=2, space="PSUM") as psT3:
                for mb in range(8):
                    tp = psT3.tile([128, 64], BF16, name="tp3")
                    nc.tensor.transpose(
                        tp, s3T_bf[:, mb * 128 : (mb + 1) * 128],
                        ident_bf[:64, :64],
                    )
                    nc.scalar.activation(s3loc[:, mb], tp, AF.Copy)
                    if mb == 3:
                        nc.sync.dma_start(out=s3_inA[:, :, :], in_=s3loc[:, 0:4])
                        nc.gpsimd.collective_compute(
                            "AllGather", mybir.AluOpType.bypass,
                            replica_groups=RG,
                            ins=[s3_inA.opt()], outs=[s3_outA.opt()],
                        )
            nc.sync.dma_start(out=s3_inB[:, :, :], in_=s3loc[:, 4:8])
            nc.gpsimd.collective_compute(
                "AllGather", mybir.AluOpType.bypass, replica_groups=RG,
                ins=[s3_inB.opt()], outs=[s3_outB.opt()],
            )

        # ========= phase D: round 3 (com = R8 @ s3q) + fusion ==========
        with tc.tile_pool(name="tailD", bufs=1) as tailD:
            comT = tailD.tile([64, NL], F32)
            zfT_sb = tailD.tile([64, NL], F32)
            nc.gpsimd.dma_start(out=zfT_sb, in_=zfT[:, :])
            zsT_sb = tailD.tile([64, NL], F32)
            nc.gpsimd.dma_start(out=zsT_sb, in_=zsT[:, :])
            wlW_sb = tailD.tile([64, 64], F32)
            nc.gpsimd.dma_start(out=wlW_sb, in_=wl_W[:, :])
            mlpW_sb = tailD.tile([64, 3, 64], F32)
            nc.gpsimd.dma_start(
                out=mlpW_sb, in_=mlp_W.rearrange("(v c) d -> c v d", v=3)
            )
            s3f = tailD.tile([128, 8, 8, Z], F8)
            with tc.tile_pool(name="psG", bufs=2, space="PSUM") as psG:
                embs = [zfT_sb, comT, zsT_sb]
                aTs = [None, None, None]
                sqs = [None, None, None]

                def emit_attn_view(v):
                    a_ps = psG.tile([64, NL], F32, name="aps")
                    for h in range(2):
                        sl = slice(h * 512, (h + 1) * 512)
                        nc.tensor.matmul(a_ps[:, sl], lhsT=wlW_sb,
                                         rhs=embs[v][:, sl])
                    aT = tailD.tile([64, NL], F32, name=f"aT{v}")
                    nc.vector.tensor_scalar_add(aT, a_ps, wl_b_sb)
                    aTs[v] = aT
                    sqv = tailD.tile([64, NL], F32, name=f"sq{v}")
                    nc.scalar.activation(sqv, aT, AF.Square)
                    sqs[v] = sqv

                with tc.tile_pool(name="psF", bufs=1, space="PSUM") as psF:
                    com_ps = psF.tile([64, NL], F32)
                    nc.scalar.dma_start(
                        out=s3f[:, :, 0:4, :],
                        in_=s3_outA.rearrange("(q p) a c -> p q a c", p=128),
                    )
                    nc.scalar.dma_start(
                        out=s3f[:, :, 4:8, :],
                        in_=s3_outB.rearrange("(q p) a c -> p q a c", p=128),
                    )
                    porder = [(q, a) for half in (0, 1) for q in range(8)
                              for a in (4 * half, 4 * half + 2)]
                    for i, (q, a) in enumerate(porder):
                        if i == 17:
                            emit_attn_view(0)
                        if i == 25:
                            emit_attn_view(2)
                        k = 8 * q + a
                        for h in range(2):
                            sl = slice(h * 512, (h + 1) * 512)
                            nc.tensor.matmul(
                                com_ps[:, sl],
                                lhsT=s3f[:, q, a : a + 2, :],
                                rhs=conT8[:, k : k + 2, sl],
                                start=(i == 0),
                                stop=(i == KC // 2 - 1),
                                perf_mode=DR,
                            )
                    nc.vector.tensor_scalar_mul(comT, com_ps, alpha_sb[:64])

                # remaining fusion (com-dependent)
                emit_attn_view(1)
                sq = tailD.tile([64, NL], F32)
                nc.vector.tensor_add(sq, sqs[0], sqs[2])
                nc.vector.tensor_add(sq, sq, sqs[1])
                nrm = tailD.tile([64, NL], F32)
                nc.scalar.activation(nrm, sq, AF.Sqrt)
                nc.vector.tensor_scalar_max(nrm, nrm, 1e-12)
                rec = tailD.tile([64, NL], F32)
                nc.vector.reciprocal_approx_fast(rec, nrm)

                out_ps = psG.tile([64, NL], F32, name="ops", bufs=1)
                for v in range(3):
                    u = tailD.tile([64, NL], F32, name="u", bufs=2)
                    nc.vector.tensor_mul(u, aTs[v], rec)
                    nc.vector.tensor_mul(u, u, embs[v])
                    for h in range(2):
                        sl = slice(h * 512, (h + 1) * 512)
                        nc.tensor.matmul(
                            out_ps[:, sl], lhsT=mlpW_sb[:, v], rhs=u[:, sl],
                            start=(v == 0), stop=(v == 2),
                        )
                outT_sb = tailD.tile([64, NL], F32)
                nc.vector.tensor_scalar_add(outT_sb, out_ps, mlp_b_sb)
                nc.gpsimd.dma_start(out=outT[:, :], in_=outT_sb)


_CACHE = {}


def _build():
    if "nc" in _CACHE:
        return _CACHE["nc"]
    nc = bacc.Bacc("TRN2", target_bir_lowering=False, debug=False,
                   num_devices=N_CORES)
    io = {
        "adjT_f": nc.dram_tensor("adjT_f", [N, NL], F8, kind="ExternalInput"),
        "adjT_s": nc.dram_tensor("adjT_s", [N, NL], F8, kind="ExternalInput"),
        "xT": nc.dram_tensor("xT", [D_IN, NL], BF16, kind="ExternalInput"),
        "zfT": nc.dram_tensor("zfT", [Z, NL], F32, kind="ExternalInput"),
        "zsT": nc.dram_tensor("zsT", [Z, NL], F32, kind="ExternalInput"),
        "W1": nc.dram_tensor("W1", [D_IN, H1], BF16, kind="ExternalInput"),
        "W2": nc.dram_tensor("W2", [H1, H2], F32, kind="ExternalInput"),
        "W3": nc.dram_tensor("W3", [H2, Z], F32, kind="ExternalInput"),
        "wl_W": nc.dram_tensor("wl_W", [Z, Z], F32, kind="ExternalInput"),
        "mlp_W": nc.dram_tensor("mlp_W", [3 * Z, Z], F32, kind="ExternalInput"),
        "wl_b": nc.dram_tensor("wl_b", [Z], F32, kind="ExternalInput"),
        "mlp_b": nc.dram_tensor("mlp_b", [Z], F32, kind="ExternalInput"),
        "meta": nc.dram_tensor("meta", [1], F32, kind="ExternalInput"),
        "outT": nc.dram_tensor("outT", [Z, NL], F32, kind="ExternalOutput"),
    }
    with tile.TileContext(nc) as tc:
        _emit(nc, tc, io)
    nc.compile()
    _CACHE["nc"] = nc
    return nc


def _shard_inputs(inputs):
    """Full inputs -> per-core input maps (host-side sharding only)."""
    f32 = np.float32
    adj_f = np.asarray(inputs["adj_feature"], f32)
    adj_s = np.asarray(inputs["adj_spatial"], f32)
    x = np.asarray(inputs["x"], f32)
    zf = np.asarray(inputs["z_feature"], f32)
    zs = np.asarray(inputs["z_spatial"], f32)
    import ml_dtypes
    bf16 = ml_dtypes.bfloat16
    fp8 = ml_dtypes.float8_e4m3fn
    rep = {
        "W1": np.ascontiguousarray(np.asarray(inputs["W1"], f32).astype(bf16)),
        "W2": np.ascontiguousarray(np.asarray(inputs["W2"], f32)),
        "W3": np.ascontiguousarray(np.asarray(inputs["W3"], f32)),
        "wl_W": np.ascontiguousarray(np.asarray(inputs["wl_W"], f32)),
        "mlp_W": np.ascontiguousarray(np.asarray(inputs["mlp_W"], f32)),
        "wl_b": np.ascontiguousarray(np.asarray(inputs["wl_b"], f32)),
        "mlp_b": np.ascontiguousarray(np.asarray(inputs["mlp_b"], f32)),
        "meta": np.ascontiguousarray(np.asarray(inputs["meta"], f32)),
    }
    # fp8 staging with a fixed 2^13 exponent shift (lossless on the
    # exponent; the mantissa rounding is the fp8 quantization itself).
    adj_fT8 = (adj_f.T * 8192.0).astype(fp8)
    adj_sT8 = (adj_s.T * 8192.0).astype(fp8)
    xT = np.ascontiguousarray(x.T)
    zfT = np.ascontiguousarray(zf.T)
    zsT = np.ascontiguousarray(zs.T)
    in_maps = []
    for i in range(N_CORES):
        r = slice(NL * i, NL * (i + 1))
        m = {
            "adjT_f": np.ascontiguousarray(adj_fT8[:, r]),
            "adjT_s": np.ascontiguousarray(adj_sT8[:, r]),
            "xT": np.ascontiguousarray(xT[:, r]).astype(bf16),
            "zfT": np.ascontiguousarray(zfT[:, r]),
            "zsT": np.ascontiguousarray(zsT[:, r]),
        }
        m.update(rep)
        in_maps.append(m)
    return in_maps


def run(trace=False, **inputs):
    nc = _build()
    in_maps = _shard_inputs(inputs)
    res = run_bass_kernel_spmd(nc, in_maps, list(range(N_CORES)), trace=trace)
    out = np.concatenate(
        [np.asarray(res.results[i]["outT"]).T for i in range(N_CORES)], axis=0
    ).astype(np.float32)
    return out, res


def kernel(**inputs):
    out, _ = run(trace=False, **inputs)
    return out


# revision 7
# speedup vs baseline: 1.7293x; 1.1211x over previous
"""Trainium2 Bass kernel for nn_CAM (GNN message passing, 8-core SPMD).

Strategy (per core i of 8, owning node rows R_i = [1024*i, 1024*(i+1))):
  - Host ships the TRANSPOSED column-block of each adjacency as
    fp8_e4m3 with a fixed 2^13 exponent shift (adj values are uniform
    [0, 1/8192]; the shift moves them into fp8's normal range and is
    folded back on-device via alpha = (1-meta)/2^13). This quarters
    the dominant HBM traffic vs f32 (8 MB + 8 MB per core).
  - x / W1 ship in bf16 and are issued at the HEAD of both DMA rings so
    the s1 = tanh(x@W1) chain and its AllGather start ~25 us in.
  - The blend  con = meta*A_f + (1-meta)*A_s  is refactored as
    con = (1-meta)/2^13 * R8  with  R8 = (c*A_f8 + A_s8),
    c = meta/(1-meta).  R8 is formed slab-by-slab with ONE fused DVE op
    (scalar_tensor_tensor, fp8 in -> fp8 out) and kept RESIDENT in SBUF
    (8 MB), so rounds 2 and 3 re-read it from SBUF instead of HBM.
  - All three adj@support rounds run as fp8 DoubleRow matmuls (2 packed
    contraction rows per PE pass = 2x throughput): lhsT is a [128,2,H]
    pair of gathered support chunks (fp8), rhs a [128,2,512] pair of
    resident R8 chunks. The support matrices are exchanged across cores
    in fp8 (AllGather bounced through shared DRAM), halving collective
    traffic; s2/s3 are exchanged as two half-collectives so the second
    half overlaps the first half's round matmuls.
  - The 2^-13*(1-meta) fold-back rides existing ACT ops for free (tanh
    input scale for s2, copy scales for s3 / com).
  - The attention fusion runs entirely in the transposed [64, 1024]
    domain; the host transposes the tiny per-core outputs back.
"""

import sys

if "/opt/trn_rl_repo" not in sys.path:
    sys.path.insert(0, "/opt/trn_rl_repo")

from contextlib import ExitStack

import numpy as np

import concourse.bass as bass
import concourse.tile as tile
from concourse import bacc, mybir
from concourse.bass_utils import run_bass_kernel_spmd
from concourse.masks import make_identity


F32 = mybir.dt.float32
BF16 = mybir.dt.bfloat16
F8 = mybir.dt.float8e4
AF = mybir.ActivationFunctionType
DR = mybir.MatmulPerfMode.DoubleRow

N = 8192
D_IN = 3000
H1, H2, Z = 256, 128, 64
N_CORES = 8
NL = N // N_CORES           # 1024 local nodes per core
KC = N // 128               # 64 contraction chunks of 128
NSLAB = 16                  # 16 slabs x 4 chunks for the adjacency stream
SCH = KC // NSLAB           # chunks per slab
XC = (D_IN + 127) // 128    # 24 x-feature chunks (last partial: 56)
RG = [list(range(N_CORES))]
INV13 = 1.0 / 8192.0        # 2^-13 fold-back for the fp8 exponent shift


def _emit(nc, tc, io):
    """Emit the whole per-core program inside a TileContext.

    Engine queues are FIFO; issue placement:
      sync ring   : meta, x/W1 even chunks, adjT_f slabs, AG input writes
      scalar ring : x/W1 odd chunks, 2 adjT_s slabs, s1 tanh chain,
                    adjT_s slabs 2.., then all tail ACT compute + reloads
      vector      : blend STTs (slab-paced), PSUM copies, fusion math
      gpsimd      : consts, AllGathers + their SBUF reloads, output
    """
    adjT_f, adjT_s, xT = io["adjT_f"], io["adjT_s"], io["xT"]
    zfT, zsT = io["zfT"], io["zsT"]
    W1, W2, W3 = io["W1"], io["W2"], io["W3"]
    wl_W, mlp_W = io["wl_W"], io["mlp_W"]
    wl_b, mlp_b, meta = io["wl_b"], io["mlp_b"], io["meta"]
    outT = io["outT"]

    ctx = ExitStack()
    with ctx:
        const = ctx.enter_context(tc.tile_pool(name="const", bufs=1))
        dram = ctx.enter_context(tc.tile_pool(name="dram", bufs=1, space="DRAM"))

        # ---- constants ----
        meta_sb = const.tile([128, 1], F32)
        nc.sync.dma_start(out=meta_sb, in_=meta.ap().to_broadcast((128, 1)))
        om_sb = const.tile([128, 1], F32)    # 1 - meta
        nc.scalar.activation(om_sb, meta_sb, AF.Copy, scale=-1.0, bias=1.0)
        alpha_sb = const.tile([128, 1], F32)  # (1 - meta) / 2^13
        nc.scalar.activation(alpha_sb, om_sb, AF.Copy, scale=INV13)
        rec_om = const.tile([128, 1], F32)   # 1 / (1 - meta)
        nc.vector.reciprocal(rec_om, om_sb)
        c_sb = const.tile([128, 1], F32)     # meta / (1 - meta)
        nc.vector.tensor_mul(c_sb, meta_sb, rec_om)

        ident_bf = const.tile([128, 128], BF16)
        make_identity(nc, ident_bf)
        wl_b_sb = const.tile([64, 1], F32)
        nc.gpsimd.dma_start(out=wl_b_sb, in_=wl_b[:, None])
        mlp_b_sb = const.tile([64, 1], F32)
        nc.gpsimd.dma_start(out=mlp_b_sb, in_=mlp_b[:, None])
        W2_sb = const.tile([128, 2, H2], BF16)
        nc.gpsimd.dma_start(out=W2_sb, in_=W2.rearrange("(b k) c -> k b c", b=2))
        W3_sb = const.tile([128, Z], BF16)
        nc.gpsimd.dma_start(out=W3_sb, in_=W3[:, :])

        # resident blended adjacency (transposed, fp8, x2^13):
        # R8[k_part, k_chunk, m]
        conT8 = const.tile([128, KC, NL], F8)
        # z1 (raw PSUM magnitude, bf16) lives across phases A->B
        z1sb = const.tile([128, 2, NL], BF16)

        # AG bounce buffers (fp8 payloads)
        s1_in = dram.tile([128, 8, H1], F8)
        s1_out = dram.tile([N // 8, 8, H1], F8, addr_space="Shared")
        s2_inA = dram.tile([128, 4, H2], F8)
        s2_outA = dram.tile([N // 8, 4, H2], F8, addr_space="Shared")
        s2_inB = dram.tile([128, 4, H2], F8)
        s2_outB = dram.tile([N // 8, 4, H2], F8, addr_space="Shared")
        s3_inA = dram.tile([128, 4, Z], F8)
        s3_outA = dram.tile([N // 8, 4, Z], F8, addr_space="Shared")
        s3_inB = dram.tile([128, 4, Z], F8)
        s3_outB = dram.tile([N // 8, 4, Z], F8, addr_space="Shared")

        # ======== phase A: stream+blend, s1 chain, round 1 ============
        with tc.tile_pool(name="phaseA", bufs=1) as pA, \
             tc.tile_pool(name="psZ", bufs=1, space="PSUM") as psZ:
            z1_ps = [psZ.tile([128, NL], F32, name=f"z1g{g}") for g in range(2)]
            s1T_bf = pA.tile([128, 2, NL], BF16)
            s1loc = pA.tile([128, 8, H1], F8)
            s1f = pA.tile([128, 8, 8, H1], F8)

            psA_ctx = ExitStack()
            psA = psA_ctx.enter_context(
                tc.tile_pool(name="psA", bufs=1, space="PSUM")
            )
            s1T_ps = [psA.tile([128, NL], F32, name=f"s1T{g}") for g in range(2)]

            # ---- x/W1 at the head of BOTH rings, alternating ----
            for kx in range(XC):
                kp = min(128, D_IN - kx * 128)
                eng = nc.sync if kx % 2 == 0 else nc.scalar
                xbf = pA.tile([128, NL], BF16, name="xbf", bufs=12)
                eng.dma_start(out=xbf[:kp], in_=xT[kx * 128 : kx * 128 + kp, :])
                w1bf = pA.tile([128, H1], BF16, name="w1bf", bufs=12)
                eng.dma_start(out=w1bf[:kp], in_=W1[kx * 128 : kx * 128 + kp, :])
                for g in range(2):
                    for h in range(2):
                        nc.tensor.matmul(
                            s1T_ps[g][:, h * 512 : (h + 1) * 512],
                            lhsT=w1bf[:kp, g * 128 : (g + 1) * 128],
                            rhs=xbf[:kp, h * 512 : (h + 1) * 512],
                            start=(kx == 0),
                            stop=(kx == XC - 1),
                        )

            # ---- s1 chain: tanh -> transposes -> fp8 copies -> AG ----
            # All ACT ops live on the scalar queue, which carries NO
            # adjacency DMAs, so the stream never stalls behind them.
            for g in range(2):
                nc.scalar.activation(s1T_bf[:, g], s1T_ps[g], AF.Tanh)
            psA_ctx.close()
            with tc.tile_pool(name="psT", bufs=2, space="PSUM") as psT:
                for mb in range(8):
                    for g in range(2):
                        tp = psT.tile([128, 128], BF16, name="tp")
                        nc.tensor.transpose(
                            tp,
                            s1T_bf[:, g, mb * 128 : (mb + 1) * 128],
                            ident_bf,
                        )
                        nc.scalar.activation(
                            s1loc[:, mb, g * 128 : (g + 1) * 128],
                            tp, AF.Copy,
                        )
            nc.scalar.dma_start(out=s1_in[:, :, :], in_=s1loc)
            nc.gpsimd.collective_compute(
                "AllGather", mybir.AluOpType.bypass,
                replica_groups=RG,
                ins=[s1_in.opt()], outs=[s1_out.opt()],
            )
            nc.gpsimd.dma_start(
                out=s1f,
                in_=s1_out.rearrange("(q p) a c -> p q a c", p=128),
            )

            # ---- adjacency slab loop (512 KB fp8 slabs, 4 chunks) ----
            # Both matrices stream on the sync ring, pairwise, so the
            # blend frontier advances in chunk order.
            for j in range(NSLAB):
                af = pA.tile([128, SCH, NL], F8, name="af", bufs=4)
                nc.sync.dma_start(
                    out=af,
                    in_=adjT_f[j * SCH * 128 : (j + 1) * SCH * 128, :].rearrange(
                        "(a p) m -> p a m", p=128
                    ),
                )
                asl = pA.tile([128, SCH, NL], F8, name="asl", bufs=4)
                nc.sync.dma_start(
                    out=asl,
                    in_=adjT_s[j * SCH * 128 : (j + 1) * SCH * 128, :].rearrange(
                        "(a p) m -> p a m", p=128
                    ),
                )
                # one fused blend per slab: R8 = (af * c) + asl -> fp8
                nc.vector.scalar_tensor_tensor(
                    out=conT8[:, j * SCH : (j + 1) * SCH, :],
                    in0=af,
                    scalar=c_sb,
                    in1=asl,
                    op0=mybir.AluOpType.mult,
                    op1=mybir.AluOpType.add,
                )

            # ---- round 1: z1 = R8 @ s1q, fp8 DoubleRow over 32 pairs ----
            for jp in range(KC // 2):
                q, a = (2 * jp) // 8, (2 * jp) % 8
                for g in range(2):
                    for h in range(2):
                        sl = slice(h * 512, (h + 1) * 512)
                        nc.tensor.matmul(
                            z1_ps[g][:, sl],
                            lhsT=s1f[:, q, a : a + 2, g * 128 : (g + 1) * 128],
                            rhs=conT8[:, 2 * jp : 2 * jp + 2, sl],
                            start=(jp == 0),
                            stop=(jp == KC // 2 - 1),
                            perf_mode=DR,
                        )
            # z1 out of PSUM (raw magnitude; alpha folds in at s2's tanh)
            nc.vector.tensor_copy(z1sb[:, 0], z1_ps[0])
            nc.scalar.copy(z1sb[:, 1], z1_ps[1])

        # ======== phase B: s2 = tanh(alpha * z1 @ W2), AG ==============
        with tc.tile_pool(name="tailB", bufs=1) as tailB:
            s2T_bf = tailB.tile([128, NL], BF16)
            s2loc = tailB.tile([128, 8, H2], F8)
            with tc.tile_pool(name="psC", bufs=1, space="PSUM") as psC:
                s2T_ps = psC.tile([128, NL], F32)
                for b in range(2):
                    for h in range(2):
                        sl = slice(h * 512, (h + 1) * 512)
                        nc.tensor.matmul(
                            s2T_ps[:, sl], lhsT=W2_sb[:, b], rhs=z1sb[:, b, sl],
                            start=(b == 0), stop=(b == 1),
                        )
                nc.scalar.activation(s2T_bf, s2T_ps, AF.Tanh, scale=alpha_sb)
            with tc.tile_pool(name="psT2", bufs=2, space="PSUM") as psT2:
                for mb in range(8):
                    tp = psT2.tile([128, 128], BF16, name="tp2")
                    nc.tensor.transpose(
                        tp, s2T_bf[:, mb * 128 : (mb + 1) * 128], ident_bf
                    )
                    nc.scalar.activation(s2loc[:, mb], tp, AF.Copy)
                    if mb == 3:
                        nc.sync.dma_start(out=s2_inA[:, :, :], in_=s2loc[:, 0:4])
                        nc.gpsimd.collective_compute(
                            "AllGather", mybir.AluOpType.bypass,
                            replica_groups=RG,
                            ins=[s2_inA.opt()], outs=[s2_outA.opt()],
                        )
            nc.sync.dma_start(out=s2_inB[:, :, :], in_=s2loc[:, 4:8])
            nc.gpsimd.collective_compute(
                "AllGather", mybir.AluOpType.bypass, replica_groups=RG,
                ins=[s2_inB.opt()], outs=[s2_outB.opt()],
            )

        # ======== phase C: round 2 (z2 = R8 @ s2q), s3 chain ===========
        with tc.tile_pool(name="tailC", bufs=1) as tailC:
            z2sb = tailC.tile([128, NL], BF16)
            s3T_bf = tailC.tile([64, NL], BF16)
            s3loc = tailC.tile([128, 8, Z], F8)
            s2f = tailC.tile([128, 8, 8, H2], F8)
            with tc.tile_pool(name="psD", bufs=1, space="PSUM") as psD:
                z2_ps = psD.tile([128, NL], F32)
                nc.scalar.dma_start(
                    out=s2f[:, :, 0:4, :],
                    in_=s2_outA.rearrange("(q p) a c -> p q a c", p=128),
                )
                nc.scalar.dma_start(
                    out=s2f[:, :, 4:8, :],
                    in_=s2_outB.rearrange("(q p) a c -> p q a c", p=128),
                )
                porder = [(q, a) for half in (0, 1) for q in range(8)
                          for a in (4 * half, 4 * half + 2)]
                for i, (q, a) in enumerate(porder):
                    k = 8 * q + a
                    for h in range(2):
                        sl = slice(h * 512, (h + 1) * 512)
                        nc.tensor.matmul(
                            z2_ps[:, sl],
                            lhsT=s2f[:, q, a : a + 2, :],
                            rhs=conT8[:, k : k + 2, sl],
                            start=(i == 0),
                            stop=(i == KC // 2 - 1),
                            perf_mode=DR,
                        )
                nc.vector.tensor_copy(z2sb[:, :512], z2_ps[:, :512])
                nc.scalar.copy(z2sb[:, 512:], z2_ps[:, 512:])

            # s3 = alpha * (z2 @ W3); fold alpha into the PSUM copy
            with tc.tile_pool(name="psE", bufs=1, space="PSUM") as psE:
                s3T_ps = psE.tile([64, NL], F32)
                for h in range(2):
                    sl = slice(h * 512, (h + 1) * 512)
                    nc.tensor.matmul(s3T_ps[:, sl], lhsT=W3_sb, rhs=z2sb[:, sl])
                nc.scalar.activation(s3T_bf, s3T_ps, AF.Copy,
                                     scale=alpha_sb[:64])
            with tc.tile_pool(name="psT3", bufs=2, space="PSUM") as psT3:
                for mb in range(8):
                    tp = psT3.tile([128, 64], BF16, name="tp3")
                    nc.tensor.transpose(
                        tp, s3T_bf[:, mb * 128 : (mb + 1) * 128],
                        ident_bf[:64, :64],
                    )
                    nc.scalar.activation(s3loc[:, mb], tp, AF.Copy)
                    if mb == 3:
                        nc.sync.dma_start(out=s3_inA[:, :, :], in_=s3loc[:, 0:4])
                        nc.gpsimd.collective_compute(
                            "AllGather", mybir.AluOpType.bypass,
                            replica_groups=RG,
                            ins=[s3_inA.opt()], outs=[s3_outA.opt()],
                        )
            nc.sync.dma_start(out=s3_inB[:, :, :], in_=s3loc[:, 4:8])
            nc.gpsimd.collective_compute(
                "AllGather", mybir.AluOpType.bypass, replica_groups=RG,
                ins=[s3_inB.opt()], outs=[s3_outB.opt()],
            )

        # ========= phase D: round 3 (com = R8 @ s3q) + fusion ==========
        with tc.tile_pool(name="tailD", bufs=1) as tailD:
            comT = tailD.tile([64, NL], F32)
            com_bf = tailD.tile([64, NL], BF16)
            zfT_sb = tailD.tile([64, NL], F32)
            nc.gpsimd.dma_start(out=zfT_sb, in_=zfT[:, :])
            zsT_sb = tailD.tile([64, NL], F32)
            nc.gpsimd.dma_start(out=zsT_sb, in_=zsT[:, :])
            zf_bf = tailD.tile([64, NL], BF16)
            nc.gpsimd.dma_start(out=zf_bf, in_=zfT[:, :])
            zs_bf = tailD.tile([64, NL], BF16)
            nc.gpsimd.dma_start(out=zs_bf, in_=zsT[:, :])
            wlW_sb = tailD.tile([64, 64], BF16)
            nc.gpsimd.dma_start(out=wlW_sb, in_=wl_W[:, :])
            mlpW_sb = tailD.tile([64, 3, 64], BF16)
            nc.gpsimd.dma_start(
                out=mlpW_sb, in_=mlp_W.rearrange("(v c) d -> c v d", v=3)
            )
            s3f = tailD.tile([128, 8, 8, Z], F8)
            with tc.tile_pool(name="psG", bufs=2, space="PSUM") as psG:
                embs = [zfT_sb, comT, zsT_sb]
                embs_bf = [zf_bf, com_bf, zs_bf]
                aTs = [None, None, None]
                sqs = [None, None, None]

                def emit_attn_view(v):
                    a_ps = psG.tile([64, NL], F32, name="aps")
                    for h in range(2):
                        sl = slice(h * 512, (h + 1) * 512)
                        nc.tensor.matmul(a_ps[:, sl], lhsT=wlW_sb,
                                         rhs=embs_bf[v][:, sl])
                    aT = tailD.tile([64, NL], F32, name=f"aT{v}")
                    nc.vector.tensor_scalar_add(aT, a_ps, wl_b_sb)
                    aTs[v] = aT
                    sqv = tailD.tile([64, NL], F32, name=f"sq{v}")
                    nc.scalar.activation(sqv, aT, AF.Square)
                    sqs[v] = sqv

                with tc.tile_pool(name="psF", bufs=1, space="PSUM") as psF:
                    com_ps = psF.tile([64, NL], F32)
                    nc.scalar.dma_start(
                        out=s3f[:, :, 0:4, :],
                        in_=s3_outA.rearrange("(q p) a c -> p q a c", p=128),
                    )
                    nc.scalar.dma_start(
                        out=s3f[:, :, 4:8, :],
                        in_=s3_outB.rearrange("(q p) a c -> p q a c", p=128),
                    )
                    porder = [(q, a) for half in (0, 1) for q in range(8)
                              for a in (4 * half, 4 * half + 2)]
                    for i, (q, a) in enumerate(porder):
                        if i == 17:
                            emit_attn_view(0)
                        if i == 25:
                            emit_attn_view(2)
                        k = 8 * q + a
                        for h in range(2):
                            sl = slice(h * 512, (h + 1) * 512)
                            nc.tensor.matmul(
                                com_ps[:, sl],
                                lhsT=s3f[:, q, a : a + 2, :],
                                rhs=conT8[:, k : k + 2, sl],
                                start=(i == 0),
                                stop=(i == KC // 2 - 1),
                                perf_mode=DR,
                            )
                    nc.vector.tensor_scalar_mul(comT, com_ps, alpha_sb[:64])
                    nc.scalar.copy(com_bf, comT)

                # remaining fusion (com-dependent)
                emit_attn_view(1)
                sq = tailD.tile([64, NL], F32)
                nc.vector.tensor_add(sq, sqs[0], sqs[2])
                nc.vector.tensor_add(sq, sq, sqs[1])
                nrm = tailD.tile([64, NL], F32)
                nc.scalar.activation(nrm, sq, AF.Sqrt)
                nc.vector.tensor_scalar_max(nrm, nrm, 1e-12)
                rec = tailD.tile([64, NL], F32)
                nc.vector.reciprocal_approx_fast(rec, nrm)

                out_ps = psG.tile([64, NL], F32, name="ops", bufs=1)
                for v in range(3):
                    ut = tailD.tile([64, NL], F32, name="ut", bufs=2)
                    nc.vector.tensor_mul(ut, aTs[v], rec)
                    u = tailD.tile([64, NL], BF16, name="u", bufs=2)
                    nc.vector.tensor_mul(u, ut, embs[v])
                    for h in range(2):
                        sl = slice(h * 512, (h + 1) * 512)
                        nc.tensor.matmul(
                            out_ps[:, sl], lhsT=mlpW_sb[:, v], rhs=u[:, sl],
                            start=(v == 0), stop=(v == 2),
                        )
                outT_sb = tailD.tile([64, NL], F32)
                nc.vector.tensor_scalar_add(outT_sb, out_ps, mlp_b_sb)
                nc.gpsimd.dma_start(out=outT[:, :], in_=outT_sb)


_CACHE = {}


def _build():
    if "nc" in _CACHE:
        return _CACHE["nc"]
    nc = bacc.Bacc("TRN2", target_bir_lowering=False, debug=False,
                   num_devices=N_CORES)
    io = {
        "adjT_f": nc.dram_tensor("adjT_f", [N, NL], F8, kind="ExternalInput"),
        "adjT_s": nc.dram_tensor("adjT_s", [N, NL], F8, kind="ExternalInput"),
        "xT": nc.dram_tensor("xT", [D_IN, NL], BF16, kind="ExternalInput"),
        "zfT": nc.dram_tensor("zfT", [Z, NL], F32, kind="ExternalInput"),
        "zsT": nc.dram_tensor("zsT", [Z, NL], F32, kind="ExternalInput"),
        "W1": nc.dram_tensor("W1", [D_IN, H1], BF16, kind="ExternalInput"),
        "W2": nc.dram_tensor("W2", [H1, H2], F32, kind="ExternalInput"),
        "W3": nc.dram_tensor("W3", [H2, Z], F32, kind="ExternalInput"),
        "wl_W": nc.dram_tensor("wl_W", [Z, Z], F32, kind="ExternalInput"),
        "mlp_W": nc.dram_tensor("mlp_W", [3 * Z, Z], F32, kind="ExternalInput"),
        "wl_b": nc.dram_tensor("wl_b", [Z], F32, kind="ExternalInput"),
        "mlp_b": nc.dram_tensor("mlp_b", [Z], F32, kind="ExternalInput"),
        "meta": nc.dram_tensor("meta", [1], F32, kind="ExternalInput"),
        "outT": nc.dram_tensor("outT", [Z, NL], F32, kind="ExternalOutput"),
    }
    with tile.TileContext(nc) as tc:
        _emit(nc, tc, io)
    nc.compile()
    _CACHE["nc"] = nc
    return nc


def _shard_inputs(inputs):
    """Full inputs -> per-core input maps (host-side sharding only)."""
    f32 = np.float32
    adj_f = np.asarray(inputs["adj_feature"], f32)
    adj_s = np.asarray(inputs["adj_spatial"], f32)
    x = np.asarray(inputs["x"], f32)
    zf = np.asarray(inputs["z_feature"], f32)
    zs = np.asarray(inputs["z_spatial"], f32)
    import ml_dtypes
    bf16 = ml_dtypes.bfloat16
    fp8 = ml_dtypes.float8_e4m3fn
    rep = {
        "W1": np.ascontiguousarray(np.asarray(inputs["W1"], f32).astype(bf16)),
        "W2": np.ascontiguousarray(np.asarray(inputs["W2"], f32)),
        "W3": np.ascontiguousarray(np.asarray(inputs["W3"], f32)),
        "wl_W": np.ascontiguousarray(np.asarray(inputs["wl_W"], f32)),
        "mlp_W": np.ascontiguousarray(np.asarray(inputs["mlp_W"], f32)),
        "wl_b": np.ascontiguousarray(np.asarray(inputs["wl_b"], f32)),
        "mlp_b": np.ascontiguousarray(np.asarray(inputs["mlp_b"], f32)),
        "meta": np.ascontiguousarray(np.asarray(inputs["meta"], f32)),
    }
    # fp8 staging with a fixed 2^13 exponent shift (lossless on the
    # exponent; the mantissa rounding is the fp8 quantization itself).
    adj_fT8 = (adj_f.T * 8192.0).astype(fp8)
    adj_sT8 = (adj_s.T * 8192.0).astype(fp8)
    xT = np.ascontiguousarray(x.T)
    zfT = np.ascontiguousarray(zf.T)
    zsT = np.ascontiguousarray(zs.T)
    in_maps = []
    for i in range(N_CORES):
        r = slice(NL * i, NL * (i + 1))
        m = {
            "adjT_f": np.ascontiguousarray(adj_fT8[:, r]),
            "adjT_s": np.ascontiguousarray(adj_sT8[:, r]),
            "xT": np.ascontiguousarray(xT[:, r]).astype(bf16),
            "zfT": np.ascontiguousarray(zfT[:, r]),
            "zsT": np.ascontiguousarray(zsT[:, r]),
        }
        m.update(rep)
        in_maps.append(m)
    return in_maps


def run(trace=False, **inputs):
    nc = _build()
    in_maps = _shard_inputs(inputs)
    res = run_bass_kernel_spmd(nc, in_maps, list(range(N_CORES)), trace=trace)
    out = np.concatenate(
        [np.asarray(res.results[i]["outT"]).T for i in range(N_CORES)], axis=0
    ).astype(np.float32)
    return out, res


def kernel(**inputs):
    out, _ = run(trace=False, **inputs)
    return out
